# revision 1
# baseline (speedup 1.0000x reference)
"""v10: raw-bacc hand-rolled synchronization — no TileContext. The kernel
is 17 instructions; Tile's exit chain (drain + 2 all-engine barriers +
sem range-clear, ~0.7us) and its conservative waits are pure overhead.
Manual sems need no cleanup: the walrus BSP postamble re-zeroes the whole
semaphore file every iteration.

Sync graph:
  sync:   DMA xw -> +16 s_x
  scalar: DMA cvec -> +16 s_c
  tensor: wait s_x; chunk0: 6 MMs (last +1 s_mm); chunk1: 6 MMs (+1 s_mm)
  vector: wait s_mm>=1, s_c; epi0; wait s_mm>=2; epi1 -> +1 s_epi
  sync:   wait s_epi; fire-and-forget DMA ot -> y (+16 s_o, never waited)
Chunk PSUM tensors sit in different banks (2KB each), so the DVE
evacuation of chunk 0 runs while the PE accumulates chunk 1.
"""

import numpy as np
import ml_dtypes

import concourse.mybir as mybir
from concourse import bacc, bass_utils

B, CIN, H, W = 8, 32, 28, 28
COUT, KH, KW = 64, 3, 3
NPIX = H * W
NCORES = 8
ROWS = H + 2
XW_COLS = ROWS * W + KH * COUT  # 1032
CHUNKS = [(0, 252), (252, 140)]
F32 = mybir.dt.float32
BF16 = mybir.dt.bfloat16

LAST_RESULTS = None
_NC = None


def _strip_const_memsets(nc):
    for fn in nc.m.functions:
        for bb in fn.blocks:
            dead = []
            for inst in bb.instructions:
                if isinstance(inst, mybir.InstMemset):
                    outs = getattr(inst, "outs", [])
                    names = [
                        getattr(getattr(o, "tensor", None), "name", "")
                        or getattr(o, "name", "")
                        or str(o)
                        for o in outs
                    ]
                    if any("const-" in n for n in names):
                        dead.append(inst)
            for inst in dead:
                bb.instructions.remove(inst)
                nc.inst_map.pop(inst.name, None)


def _build_bass():
    nc = bacc.Bacc("TRN2", debug=False, enable_asserts=False, num_devices=NCORES)
    xw = nc.dram_tensor("xw", [96, XW_COLS], BF16, kind="ExternalInput")
    cv = nc.dram_tensor("cvec", [128, 1], F32, kind="ExternalInput")
    y = nc.dram_tensor("y", [128, 392], F32, kind="ExternalOutput")

    xt = nc.alloc_sbuf_tensor("xt", [96, XW_COLS], BF16)
    ct = nc.alloc_sbuf_tensor("ct", [128, 1], F32)
    ot = nc.alloc_sbuf_tensor("ot", [128, 392], F32)
    ps0 = nc.alloc_psum_tensor("ps0", [128, 512], F32)
    ps1 = nc.alloc_psum_tensor("ps1", [128, 512], F32)

    s_x = nc.alloc_semaphore("s_x")
    s_c = nc.alloc_semaphore("s_c")
    s_mm = nc.alloc_semaphore("s_mm")
    s_epi = nc.alloc_semaphore("s_epi")
    s_o = nc.alloc_semaphore("s_o")

    nc.sync.dma_start(xt.ap(), xw.ap()).then_inc(s_x, 16)
    nc.scalar.dma_start(ct.ap(), cv.ap()).then_inc(s_c, 16)

    wof = ROWS * W
    nc.tensor.wait_ge(s_x, 16)
    for c, (coff, cw) in enumerate(CHUNKS):
        ps = (ps0 if c == 0 else ps1).ap()[:, :cw]
        for ki in range(KH):
            for h in range(2):
                off = ki * W + h * 392 + coff
                mm = nc.tensor.matmul(
                    ps[h * COUT : (h + 1) * COUT, :],
                    xt.ap()[:, wof + ki * COUT : wof + (ki + 1) * COUT],
                    xt.ap()[:, off : off + cw],
                    start=(ki == 0),
                    stop=(ki == KH - 1),
                    skip_group_check=True,
                )
        mm.then_inc(s_mm, 1)  # MMs complete in pc order; last covers chunk

    nc.vector.wait_ge(s_c, 16)
    nc.vector.wait_ge(s_mm, 1)
    nc.vector.tensor_scalar_add(
        ot.ap()[:, 0 : CHUNKS[0][1]], ps0.ap()[:, : CHUNKS[0][1]], ct.ap()
    )
    nc.vector.wait_ge(s_mm, 2)
    nc.vector.tensor_scalar_add(
        ot.ap()[:, CHUNKS[1][0] : 392], ps1.ap()[:, : CHUNKS[1][1]], ct.ap()
    ).then_inc(s_epi, 1)

    nc.sync.wait_ge(s_epi, 1)
    nc.sync.dma_start(y.ap(), ot.ap()).then_inc(s_o, 16)

    _strip_const_memsets(nc)
    nc.finalize()
    return nc


def _get_nc():
    global _NC
    if _NC is None:
        _NC = _build_bass()
    return _NC


def _host_prep(x, k, bias, delta_x, delta_w):
    kf = k.reshape(KH * KW * CIN, COUT).astype(np.float64)
    wexp = np.exp(kf + 5.0)
    wmod = (wexp - float(delta_w)).astype(np.float32)
    cvec = (
        wexp.sum(axis=0)
        - float(delta_x) * kf.sum(axis=0)
        + bias.astype(np.float64)
    ).astype(np.float32)

    wdev = (
        wmod.reshape(KH, KW * CIN, COUT).transpose(1, 0, 2).reshape(96, KH * COUT)
    )
    cv2 = np.ascontiguousarray(np.concatenate([cvec, cvec]).reshape(128, 1))

    xpad = np.zeros((B, CIN, ROWS, W + 2), np.float32)
    xpad[:, :, 1 : H + 1, 1 : W + 1] = x
    xblk = np.stack([xpad[:, :, :, kj : kj + W] for kj in range(KW)], axis=1)
    xbs = xblk.reshape(B, KW * CIN, ROWS * W)
    xw = np.concatenate([xbs, np.broadcast_to(wdev, (B, 96, KH * COUT))], axis=2)
    xw_in = np.ascontiguousarray(xw.astype(ml_dtypes.bfloat16))
    return xw_in, cv2


def _unshuffle(yarr):
    yv = yarr.reshape(2, COUT, 392)
    return np.concatenate([yv[0], yv[1]], axis=1)


def _in_maps(x, k, bias, delta_x, delta_w):
    xw_in, cv2 = _host_prep(x, k, bias, delta_x, delta_w)
    return [{"xw": xw_in[b], "cvec": cv2} for b in range(NCORES)]


def kernel(x, k, bias, delta_x, delta_w):
    global LAST_RESULTS
    x = np.ascontiguousarray(np.asarray(x, dtype=np.float32))
    k = np.asarray(k, dtype=np.float32)
    bias = np.asarray(bias, dtype=np.float32)

    in_maps = _in_maps(x, k, bias, delta_x, delta_w)
    nc = _get_nc()
    res = bass_utils.run_bass_kernel_spmd(nc, in_maps, core_ids=list(range(NCORES)))
    LAST_RESULTS = res
    out = np.stack(
        [_unshuffle(res.results[b]["y"]).reshape(COUT, H, W) for b in range(B)]
    )
    return out.astype(np.float32)

